# revision 17
# baseline (speedup 1.0000x reference)
"""Distributed multi-head attention kernel for 8 TRN2 NeuronCores.

Sharding: 8-way head parallel (2 heads per core), batches looped on-core.
Each core: QKV projection for its 2 heads over both batches, per-head
attention (softmax without max-subtraction — logits are small; denominators
come from a ones-column appended to V so they fall out of the attn@V
matmul), then per-head AllToAlls across all 8 cores exchange row-blocks
for head-blocks: block s = (batch s//4, rows-block s%4).  Core c ends up
with all 1024 inner dims for (batch c//4, rows [(c%4)*512, ...)) and runs
the full output projection + bias on that slice.

The attention phase is ACT-bound (the exp stream, ~1.13us per 2-chunk
group).  Emission is software-pipelined so the PE always has work during
the exp lag: each block's QK^T dots are emitted ahead of the previous
block's attn@V, and the gaps are filled with the other batch's
projections, the head-0 A2A receive, and the first half of the output
projection.  The output projection is split into two half-contraction
passes: attnT contraction chunks are laid out as (local_head*4 + i//2)
with partition offset (i%2)*64 (W_out rows permuted to match on the
host), so chunks 0-3 are complete after receive(0) and that half of the
projection runs while the head-1 A2A is in flight.

x and the weights are cast to bf16 on the host (bf16 is the compute
precision anyway) and x additionally arrives pre-transposed, so x^T
streams straight into SBUF with fully contiguous DMAs.  The A2A-receive
transposes run on the PE array.  The scalar queue carries only the exp
activations; all DMAs are issued from the sync/gpsimd queues.

The per-core output is the TRANSPOSED final slice [1024, 512] (PSUM-major
writes stay contiguous); the host transposes during assembly.
"""
import numpy as np

import concourse.bass as bass
import concourse.mybir as mybir
from concourse import bacc
import concourse.tile as tile
from concourse.bass_utils import run_bass_kernel_spmd
from concourse.masks import make_identity

# problem constants (hardcoded; kernel.py must be self-contained)
B, N, DIM = 2, 2048, 1024
H, DH = 16, 64
INNER = H * DH            # 1024
SCALE = DIM ** -0.5       # 1/32  (module scales by dim**-0.5, not dim_head)
NCORES = 8
HPC = H // NCORES         # 2 heads per core
SH = HPC * DH             # 128 inner cols per core
ROWS = N // 4             # 512 output rows per core
P = 128
KO = DIM // P             # 8 contraction chunks
JC = N // P               # 16 row chunks
IB = 512                  # query block size
NIB = N // IB             # 4 query blocks
ISUB = IB // P            # 4
FP32 = mybir.dt.float32
BF16 = mybir.dt.bfloat16

REPLICA_GROUPS = [[0, 1, 2, 3, 4, 5, 6, 7]]

_NC_CACHE = {}

# set by the last kernel() call when BASS_KERNEL_TRACE=1 (for test.py)
LAST_RESULTS = None


def _build():
    nc = bacc.Bacc(num_devices=NCORES)

    x_ext = nc.declare_dram_parameter("x", [B * DIM, N], BF16, isOutput=False)
    wq_ext = nc.declare_dram_parameter("wq", [DIM, SH], BF16, isOutput=False)
    wk_ext = nc.declare_dram_parameter("wk", [DIM, SH], BF16, isOutput=False)
    wv_ext = nc.declare_dram_parameter("wv", [DIM, SH], BF16, isOutput=False)
    wo_ext = nc.declare_dram_parameter("wo", [DIM, DIM], BF16, isOutput=False)
    bo_ext = nc.declare_dram_parameter("bo", [DIM], FP32, isOutput=False)
    out_ext = nc.declare_dram_parameter("out", [DIM, ROWS], FP32, isOutput=True)

    with tile.TileContext(nc) as tc:
        with (
            tc.tile_pool(name="consts", bufs=1) as consts,
            tc.tile_pool(name="stage", bufs=2) as stage,
            tc.tile_pool(name="xt_pool", bufs=2) as xt_pool,
            tc.tile_pool(name="pt_pool", bufs=3) as pt_pool,
            tc.tile_pool(name="nrm", bufs=8) as nrm,
            tc.tile_pool(name="rst_pool", bufs=8) as rst_pool,
            tc.tile_pool(name="dram", bufs=1, space="DRAM") as dram,
            tc.tile_pool(name="st_psum", bufs=3, space="PSUM") as st_psum,
            tc.tile_pool(name="o_psum", bufs=2, space="PSUM") as o_psum,
        ):
            ident_bf = consts.tile([P, P], BF16)

            # persistent tensors
            wq_sb = consts.tile([P, KO, SH], BF16)
            wk_sb = consts.tile([P, KO, SH], BF16)
            wv_sb = consts.tile([P, KO, SH], BF16)
            wo_sb = consts.tile([P, KO, DIM], BF16)
            bias_sb = consts.tile([P, KO], FP32)
            qT = consts.tile([P, B, HPC, N], BF16)   # [d(+zero pad), b, h, i]
            kT = consts.tile([P, B, HPC, N], BF16)
            v_aug = consts.tile([P, B, JC, HPC, DH + 1], BF16)
            out_rows = consts.tile([P, B, JC, SH], BF16)
            fstage = consts.tile([P, KO, ROWS], BF16)  # half-done out proj
            attnT = xt_pool.tile([P, KO, N], BF16, tag="xT", name="attnT")[
                :, :, :ROWS
            ]

            a2a_in0 = dram.tile([NCORES, P, NIB, DH], BF16, name="a2a_in0")
            a2a_in1 = dram.tile([NCORES, P, NIB, DH], BF16, name="a2a_in1")
            a2a_out0 = dram.tile([NCORES, P, NIB, DH], BF16, name="a2a_out0")
            a2a_out1 = dram.tile([NCORES, P, NIB, DH], BF16, name="a2a_out1")
            a2a_ins = [a2a_in0, a2a_in1]
            a2a_outs = [a2a_out0, a2a_out1]

            nc.vector.memset(v_aug[:, :, :, :, DH : DH + 1], 1.0)

            def load_xT(b):
                """x[b]^T arrives pre-transposed from the host: straight
                contiguous loads; one dma_start per (nb, ko) chunk so the
                transfers spread across DMA rings."""
                xT = xt_pool.tile([P, KO, N], BF16, tag="xT", name="xT")
                for nb in range(NIB):
                    for ko in range(KO):
                        nc.sync.dma_start(
                            xT[:, ko, nb * IB : (nb + 1) * IB],
                            x_ext[
                                b * DIM + ko * P : b * DIM + (ko + 1) * P,
                                nb * IB : (nb + 1) * IB,
                            ],
                        )
                return xT

            def load_qkv_weights():
                for w_ext, w_sb in (
                    (wk_ext, wk_sb),
                    (wq_ext, wq_sb),
                    (wv_ext, wv_sb),
                ):
                    nc.gpsimd.dma_start(
                        w_sb, w_ext.rearrange("(ko kp) c -> kp ko c", kp=P)
                    )

            def load_out_weights():
                nc.gpsimd.dma_start(
                    wo_sb, wo_ext.rearrange("(ko kp) c -> kp ko c", kp=P)
                )
                nc.gpsimd.dma_start(
                    bias_sb, bo_ext.rearrange("(co cp) -> cp co", cp=P)
                )

            def qk_group(b, which, nb, xT):
                """One accumulation group of the q or k projection."""
                w_sb, dstT = (
                    (wk_sb, kT) if which == "k" else (wq_sb, qT)
                )
                ps2 = st_psum.tile([P, 2, IB], FP32, tag="st", name="qk_ps")
                ps = ps2[:, 0, :]
                for ko in range(KO):
                    nc.tensor.matmul(
                        ps,
                        w_sb[:, ko, :],
                        xT[:, ko, nb * IB : (nb + 1) * IB],
                        start=(ko == 0),
                        stop=(ko == KO - 1),
                    )
                for h in range(HPC):
                    nc.vector.tensor_copy(
                        dstT[0:DH, b, h, nb * IB : (nb + 1) * IB],
                        ps[h * DH : (h + 1) * DH, :],
                    )

            def v_group(b, mt, xT):
                psv2 = st_psum.tile([P, 2, IB], FP32, tag="st", name="v_ps")
                ps_v = psv2[:, 0, :SH]
                for ko in range(KO):
                    nc.tensor.matmul(
                        ps_v,
                        xT[:, ko, mt * P : (mt + 1) * P],
                        wv_sb[:, ko, :],
                        start=(ko == 0),
                        stop=(ko == KO - 1),
                    )
                nc.vector.tensor_copy(
                    v_aug[:, b, mt, :, 0:DH],
                    ps_v.rearrange("p (h d) -> p h d", d=DH),
                )

            def dots_block(h, b, ib):
                """QK^T + exp for one query block; returns the ptile for
                the matching attnv_block call."""
                ptile = pt_pool.tile([P, JC, IB], BF16, tag="pt", name="ptile")
                for jg in range(JC // 2):
                    ps_st = st_psum.tile(
                        [P, 2, IB], FP32, tag="st", name="st_ps"
                    )
                    for u in range(2):
                        jc = jg * 2 + u
                        nc.tensor.matmul(
                            ps_st[:, u, :],
                            kT[:, b, h, jc * P : (jc + 1) * P],
                            qT[:, b, h, ib * IB : (ib + 1) * IB],
                            start=True,
                            stop=True,
                        )
                    nc.scalar.activation(
                        ptile[:, jg * 2 : (jg + 1) * 2, :],
                        ps_st,
                        mybir.ActivationFunctionType.Exp,
                        scale=SCALE,
                    )
                return ptile

            def attnv_block(h, b, ib, ptile):
                """attn@V with the denominator in column DH, then normalize
                and stage this block's A2A input."""
                po = h * DH
                ps_o4 = o_psum.tile(
                    [P, ISUB, DH + 1], FP32, tag="po", name="o_ps"
                )
                for isub in range(ISUB):
                    for jc in range(JC):
                        nc.tensor.matmul(
                            ps_o4[:, isub, :],
                            ptile[:, jc, isub * P : (isub + 1) * P],
                            v_aug[:, b, jc, h, :],
                            start=(jc == 0),
                            stop=(jc == JC - 1),
                        )
                for isub in range(ISUB):
                    ic = ib * ISUB + isub
                    recip = nrm.tile([P, 1], FP32, tag="recip", name="recip")
                    nc.vector.reciprocal(recip, ps_o4[:, isub, DH : DH + 1])
                    nc.vector.tensor_scalar_mul(
                        out_rows[:, b, ic, po : po + DH],
                        ps_o4[:, isub, 0:DH],
                        recip,
                    )
                s = b * NIB + ib
                nc.sync.dma_start(
                    a2a_ins[h][s],
                    out_rows[
                        :, b, ib * ISUB : (ib + 1) * ISUB, po : po + DH
                    ],
                )

            def a2a_exchange(h):
                nc.gpsimd.collective_compute(
                    "AllToAll",
                    mybir.AluOpType.bypass,
                    replica_groups=REPLICA_GROUPS,
                    ins=[a2a_ins[h].opt()],
                    outs=[a2a_outs[h].opt()],
                )

            def receive(hl, i_list):
                """Unpack a2a_outs[hl] blocks from source cores i_list into
                attnT chunk (hl*4 + i//2) at partition offset (i%2)*64."""
                rstages = {}
                for i in i_list:
                    rstage = rst_pool.tile(
                        [P, NIB, DH], BF16, tag="rstage", name="rstage"
                    )
                    nc.sync.dma_start(rstage, a2a_outs[hl][i])
                    rstages[i] = rstage
                for i in i_list:
                    rps = st_psum.tile([DH, NIB, P], BF16, tag="st", name="r_ps")
                    for q in range(NIB):
                        nc.tensor.transpose(
                            rps[:, q, :], rstages[i][:, q, :], ident_bf
                        )
                    pb = (i % 2) * DH
                    nc.vector.tensor_copy(
                        attnT[pb : pb + DH, hl * 4 + i // 2, :], rps
                    )

            def final_half_a(cc):
                """Output projection over attnT chunks 0-3 (head-local 0),
                bias folded in; parked in fstage."""
                psf2 = st_psum.tile([P, 2, IB], FP32, tag="st", name="fa_ps")
                ps_f = psf2[:, 0, :ROWS]
                for ko in range(4):
                    nc.tensor.matmul(
                        ps_f,
                        wo_sb[:, ko, cc * P : (cc + 1) * P],
                        attnT[:, ko, :],
                        start=(ko == 0),
                        stop=(ko == 3),
                    )
                nc.vector.tensor_scalar_add(
                    fstage[:, cc, :], ps_f, bias_sb[:, cc : cc + 1]
                )

            def final_half_b(cc):
                """Chunks 4-7 (head-local 1), summed with the parked half."""
                psf2 = st_psum.tile([P, 2, IB], FP32, tag="st", name="fb_ps")
                ps_f = psf2[:, 0, :ROWS]
                for ko in range(4, KO):
                    nc.tensor.matmul(
                        ps_f,
                        wo_sb[:, ko, cc * P : (cc + 1) * P],
                        attnT[:, ko, :],
                        start=(ko == 4),
                        stop=(ko == KO - 1),
                    )
                of = stage.tile([P, ROWS], FP32, tag="of", name="of")
                nc.vector.tensor_tensor(
                    of, ps_f, fstage[:, cc, :], mybir.AluOpType.add
                )
                nc.sync.dma_start(out_ext[cc * P : (cc + 1) * P, :], of)

            # ---- emission order: software-pipelined so the exp stream
            # starts early and never starves, with projection / receive /
            # half-projection work filling the PE's exp-lag bubbles ----
            # gpsimd queue order matters at startup: the qkv weight DMAs
            # must precede the big pad memsets (each ~6-10us on gpsimd) or
            # the first qk matmul waits ~20us for wk to land
            load_qkv_weights()
            nc.gpsimd.memset(qT[DH:P, :, :, :], 0.0)
            nc.gpsimd.memset(kT[DH:P, :, :, :], 0.0)
            xT0 = load_xT(0)
            xT1 = load_xT(1)
            load_out_weights()
            make_identity(nc, ident_bf)

            for nb in range(NIB):
                qk_group(0, "k", nb, xT0)
            qk_group(0, "q", 0, xT0)

            pt = {}
            pt[(0, 0, 0)] = dots_block(0, 0, 0)
            qk_group(0, "q", 1, xT0)
            qk_group(0, "q", 2, xT0)
            pt[(0, 0, 1)] = dots_block(0, 0, 1)
            qk_group(0, "q", 3, xT0)
            for mt in range(8):
                v_group(0, mt, xT0)
            pt[(0, 0, 2)] = dots_block(0, 0, 2)
            for mt in range(8, JC):
                v_group(0, mt, xT0)
            pt[(0, 0, 3)] = dots_block(0, 0, 3)

            attnv_block(0, 0, 0, pt[(0, 0, 0)])
            qk_group(1, "k", 0, xT1)
            qk_group(1, "k", 1, xT1)
            attnv_block(0, 0, 1, pt[(0, 0, 1)])
            qk_group(1, "k", 2, xT1)
            qk_group(1, "k", 3, xT1)
            attnv_block(0, 0, 2, pt[(0, 0, 2)])
            qk_group(1, "q", 0, xT1)
            pt[(0, 1, 0)] = dots_block(0, 1, 0)
            attnv_block(0, 0, 3, pt[(0, 0, 3)])
            qk_group(1, "q", 1, xT1)
            pt[(0, 1, 1)] = dots_block(0, 1, 1)
            for mt in range(8):
                v_group(1, mt, xT1)
            attnv_block(0, 1, 0, pt[(0, 1, 0)])
            qk_group(1, "q", 2, xT1)
            pt[(0, 1, 2)] = dots_block(0, 1, 2)
            for mt in range(8, JC):
                v_group(1, mt, xT1)
            attnv_block(0, 1, 1, pt[(0, 1, 1)])
            qk_group(1, "q", 3, xT1)
            pt[(0, 1, 3)] = dots_block(0, 1, 3)
            attnv_block(0, 1, 2, pt[(0, 1, 2)])
            attnv_block(0, 1, 3, pt[(0, 1, 3)])
            a2a_exchange(0)

            pt[(1, 0, 0)] = dots_block(1, 0, 0)
            pt[(1, 0, 1)] = dots_block(1, 0, 1)
            attnv_block(1, 0, 0, pt[(1, 0, 0)])
            pt[(1, 0, 2)] = dots_block(1, 0, 2)
            attnv_block(1, 0, 1, pt[(1, 0, 1)])
            pt[(1, 0, 3)] = dots_block(1, 0, 3)
            attnv_block(1, 0, 2, pt[(1, 0, 2)])
            attnv_block(1, 0, 3, pt[(1, 0, 3)])

            pt[(1, 1, 0)] = dots_block(1, 1, 0)
            pt[(1, 1, 1)] = dots_block(1, 1, 1)
            attnv_block(1, 1, 0, pt[(1, 1, 0)])
            pt[(1, 1, 2)] = dots_block(1, 1, 2)
            attnv_block(1, 1, 1, pt[(1, 1, 1)])
            pt[(1, 1, 3)] = dots_block(1, 1, 3)
            attnv_block(1, 1, 2, pt[(1, 1, 2)])
            attnv_block(1, 1, 3, pt[(1, 1, 3)])
            a2a_exchange(1)
            # head-0 receive + first half of the output projection fill
            # the PE while the head-1 A2A is in flight
            receive(0, [0, 1, 2, 3])
            receive(0, [4, 5, 6, 7])
            for cc in range(KO):
                final_half_a(cc)
            receive(1, list(range(NCORES)))
            for cc in range(KO):
                final_half_b(cc)

    nc.finalize()
    return nc


def _get_nc():
    if "nc" not in _NC_CACHE:
        _NC_CACHE["nc"] = _build()
    return _NC_CACHE["nc"]


def _wo_row_perm():
    """attnT contraction chunk ko holds (local head ko//4, source-core
    pair ko%4) with partition p = (i%2)*64 + d; W_out rows are permuted
    to match so wo_sb[(ko, p)] multiplies the right inner dim."""
    perm = np.empty(DIM, dtype=np.int64)
    for ko in range(KO):
        hl, ipair = ko // 4, ko % 4
        for p_ in range(P):
            i = ipair * 2 + p_ // DH
            d = p_ % DH
            perm[ko * P + p_] = (2 * i + hl) * DH + d
    return perm


def kernel(**inputs) -> np.ndarray:
    import os

    import ml_dtypes

    global LAST_RESULTS

    bf16 = ml_dtypes.bfloat16
    x = np.asarray(inputs["x"], dtype=np.float32)
    W_qkv = np.asarray(inputs["W_qkv"], dtype=np.float32)
    W_out = np.asarray(inputs["W_out"], dtype=np.float32)
    b_out = np.ascontiguousarray(np.asarray(inputs["b_out"], dtype=np.float32))

    x_bf = np.ascontiguousarray(
        x.transpose(0, 2, 1).reshape(B * DIM, N).astype(bf16)
    )
    wo_bf = np.ascontiguousarray(W_out.astype(bf16)[_wo_row_perm(), :])
    wqkv_bf = W_qkv.astype(bf16)

    nc = _get_nc()

    in_maps = []
    for c in range(NCORES):
        in_maps.append(
            {
                "x": x_bf,
                "wq": np.ascontiguousarray(
                    wqkv_bf[:, 0 * INNER + c * SH : 0 * INNER + (c + 1) * SH]
                ),
                "wk": np.ascontiguousarray(
                    wqkv_bf[:, 1 * INNER + c * SH : 1 * INNER + (c + 1) * SH]
                ),
                "wv": np.ascontiguousarray(
                    wqkv_bf[:, 2 * INNER + c * SH : 2 * INNER + (c + 1) * SH]
                ),
                "wo": wo_bf,
                "bo": b_out,
            }
        )

    trace = os.environ.get("BASS_KERNEL_TRACE", "0") == "1"
    res = run_bass_kernel_spmd(
        nc, in_maps, core_ids=list(range(NCORES)), trace=trace
    )
    LAST_RESULTS = res

    y = np.empty((B, N, DIM), dtype=np.float32)
    for c in range(NCORES):
        b, r = c // 4, c % 4
        y[b, r * ROWS : (r + 1) * ROWS, :] = res.results[c]["out"].T
    return y


# revision 22
# speedup vs baseline: 1.0569x; 1.0569x over previous
"""Distributed multi-head attention kernel for 8 TRN2 NeuronCores.

Sharding: 8-way head parallel (2 heads per core), batches looped on-core.
Each core: QKV projection for its 2 heads over both batches, per-head
attention (softmax without max-subtraction — logits are small; denominators
come from a ones-column appended to V so they fall out of the attn@V
matmul), then per-head AllToAlls across all 8 cores exchange row-blocks
for head-blocks: block s = (batch s//4, rows-block s%4).  Core c ends up
with all 1024 inner dims for (batch c//4, rows [(c%4)*512, ...)) and runs
the full output projection + bias on that slice.

The attention phase is ACT-bound (the exp stream, ~1.13us per 2-chunk
group).  Emission is software-pipelined so the PE always has work during
the exp lag: each block's QK^T dots are emitted ahead of the previous
block's attn@V, and the gaps are filled with the other batch's
projections, the head-0 A2A receive, and the first half of the output
projection.  The output projection is split into two half-contraction
passes: attnT contraction chunks are laid out as (local_head*4 + i//2)
with partition offset (i%2)*64 (W_out rows permuted to match on the
host), so chunks 0-3 are complete after receive(0) and that half of the
projection runs while the head-1 A2A is in flight.

x and the weights are cast to bf16 on the host (bf16 is the compute
precision anyway) and x additionally arrives pre-transposed, so x^T
streams straight into SBUF with fully contiguous DMAs.  The A2A-receive
transposes run on the PE array.  The scalar queue carries only the exp
activations; all DMAs are issued from the sync/gpsimd queues.

The per-core output is the TRANSPOSED final slice [1024, 512] (PSUM-major
writes stay contiguous); the host transposes during assembly.
"""
import numpy as np

import concourse.bass as bass
import concourse.mybir as mybir
from concourse import bacc
import concourse.tile as tile
from concourse.bass_utils import run_bass_kernel_spmd
from concourse.masks import make_identity

# problem constants (hardcoded; kernel.py must be self-contained)
B, N, DIM = 2, 2048, 1024
H, DH = 16, 64
INNER = H * DH            # 1024
SCALE = DIM ** -0.5       # 1/32  (module scales by dim**-0.5, not dim_head)
NCORES = 8
HPC = H // NCORES         # 2 heads per core
SH = HPC * DH             # 128 inner cols per core
ROWS = N // 4             # 512 output rows per core
P = 128
KO = DIM // P             # 8 contraction chunks
JC = N // P               # 16 row chunks
IB = 512                  # query block size
NIB = N // IB             # 4 query blocks
ISUB = IB // P            # 4
FP32 = mybir.dt.float32
BF16 = mybir.dt.bfloat16

REPLICA_GROUPS = [[0, 1, 2, 3, 4, 5, 6, 7]]

_NC_CACHE = {}

# set by the last kernel() call when BASS_KERNEL_TRACE=1 (for test.py)
LAST_RESULTS = None


def _build():
    nc = bacc.Bacc(num_devices=NCORES)

    # weights arrive pre-rearranged from the host ([kp, ko, c] layout) so
    # their DMAs are fully contiguous — the strided rearrange gather cost
    # ~4.5us on the critical path at startup
    x_ext = nc.declare_dram_parameter("x", [B * DIM, N], BF16, isOutput=False)
    wq_ext = nc.declare_dram_parameter("wq", [P, KO * SH], BF16, isOutput=False)
    wk_ext = nc.declare_dram_parameter("wk", [P, KO * SH], BF16, isOutput=False)
    wv_ext = nc.declare_dram_parameter("wv", [P, KO * SH], BF16, isOutput=False)
    wo_ext = nc.declare_dram_parameter("wo", [P, KO * DIM], BF16, isOutput=False)
    bo_ext = nc.declare_dram_parameter("bo", [P, KO], FP32, isOutput=False)
    out_ext = nc.declare_dram_parameter("out", [DIM, ROWS], FP32, isOutput=True)

    with tile.TileContext(nc) as tc:
        with (
            tc.tile_pool(name="consts", bufs=1) as consts,
            tc.tile_pool(name="stage", bufs=2) as stage,
            tc.tile_pool(name="xt_pool", bufs=2) as xt_pool,
            tc.tile_pool(name="pt_pool", bufs=3) as pt_pool,
            tc.tile_pool(name="nrm", bufs=8) as nrm,
            tc.tile_pool(name="rst_pool", bufs=8) as rst_pool,
            tc.tile_pool(name="dram", bufs=1, space="DRAM") as dram,
            tc.tile_pool(name="st_psum", bufs=3, space="PSUM") as st_psum,
            tc.tile_pool(name="o_psum", bufs=2, space="PSUM") as o_psum,
        ):
            ident_bf = consts.tile([P, P], BF16)

            # persistent tensors
            wq_sb = consts.tile([P, KO, SH], BF16)
            wk_sb = consts.tile([P, KO, SH], BF16)
            wv_sb = consts.tile([P, KO, SH], BF16)
            wo_sb = consts.tile([P, KO, DIM], BF16)
            bias_sb = consts.tile([P, KO], FP32)
            qT = consts.tile([P, B, HPC, N], BF16)   # [d(+zero pad), b, h, i]
            kT = consts.tile([P, B, HPC, N], BF16)
            v_aug = consts.tile([P, B, JC, HPC, DH + 1], BF16)
            out_rows = consts.tile([P, B, JC, SH], BF16)
            fstage = consts.tile([P, KO, ROWS], BF16)  # half-done out proj
            attnT = xt_pool.tile([P, KO, N], BF16, tag="xT", name="attnT")[
                :, :, :ROWS
            ]

            a2a_in0 = dram.tile([NCORES, P, NIB, DH], BF16, name="a2a_in0")
            a2a_in1 = dram.tile([NCORES, P, NIB, DH], BF16, name="a2a_in1")
            a2a_out0 = dram.tile([NCORES, P, NIB, DH], BF16, name="a2a_out0")
            a2a_out1 = dram.tile([NCORES, P, NIB, DH], BF16, name="a2a_out1")
            a2a_ins = [a2a_in0, a2a_in1]
            a2a_outs = [a2a_out0, a2a_out1]

            nc.vector.memset(v_aug[:, :, :, :, DH : DH + 1], 1.0)

            def load_xT(b, engines=(nc.sync,)):
                """x[b]^T arrives pre-transposed from the host: straight
                contiguous loads, one dma_start per (nb, ko) chunk.  DMA
                issue costs ~0.63us each on an engine queue, so the first
                batch alternates chunks across two queues to halve the
                delivery cadence the qk projection rides on."""
                xT = xt_pool.tile([P, KO, N], BF16, tag="xT", name="xT")
                idx = 0
                for nb in range(NIB):
                    for ko in range(KO):
                        eng = engines[idx % len(engines)]
                        idx += 1
                        eng.dma_start(
                            xT[:, ko, nb * IB : (nb + 1) * IB],
                            x_ext[
                                b * DIM + ko * P : b * DIM + (ko + 1) * P,
                                nb * IB : (nb + 1) * IB,
                            ],
                        )
                return xT

            def load_qkv_weights():
                for w_ext, w_sb in (
                    (wk_ext, wk_sb),
                    (wq_ext, wq_sb),
                    (wv_ext, wv_sb),
                ):
                    nc.gpsimd.dma_start(
                        w_sb, w_ext.rearrange("kp (ko c) -> kp ko c", ko=KO)
                    )

            def load_out_weights():
                nc.gpsimd.dma_start(
                    wo_sb, wo_ext.rearrange("kp (ko c) -> kp ko c", ko=KO)
                )
                nc.gpsimd.dma_start(bias_sb, bo_ext[:, :])

            def qk_group(b, which, nb, xT):
                """One accumulation group of the q or k projection."""
                w_sb, dstT = (
                    (wk_sb, kT) if which == "k" else (wq_sb, qT)
                )
                ps2 = st_psum.tile([P, 2, IB], FP32, tag="st", name="qk_ps")
                ps = ps2[:, 0, :]
                for ko in range(KO):
                    nc.tensor.matmul(
                        ps,
                        w_sb[:, ko, :],
                        xT[:, ko, nb * IB : (nb + 1) * IB],
                        start=(ko == 0),
                        stop=(ko == KO - 1),
                    )
                for h in range(HPC):
                    nc.vector.tensor_copy(
                        dstT[0:DH, b, h, nb * IB : (nb + 1) * IB],
                        ps[h * DH : (h + 1) * DH, :],
                    )

            def v_group(b, mt, xT):
                psv2 = st_psum.tile([P, 2, IB], FP32, tag="st", name="v_ps")
                ps_v = psv2[:, 0, :SH]
                for ko in range(KO):
                    nc.tensor.matmul(
                        ps_v,
                        xT[:, ko, mt * P : (mt + 1) * P],
                        wv_sb[:, ko, :],
                        start=(ko == 0),
                        stop=(ko == KO - 1),
                    )
                nc.vector.tensor_copy(
                    v_aug[:, b, mt, :, 0:DH],
                    ps_v.rearrange("p (h d) -> p h d", d=DH),
                )

            def dots_block(h, b, ib):
                """QK^T + exp for one query block; returns the ptile for
                the matching attnv_block call."""
                ptile = pt_pool.tile([P, JC, IB], BF16, tag="pt", name="ptile")
                for jg in range(JC // 2):
                    ps_st = st_psum.tile(
                        [P, 2, IB], FP32, tag="st", name="st_ps"
                    )
                    for u in range(2):
                        jc = jg * 2 + u
                        nc.tensor.matmul(
                            ps_st[:, u, :],
                            kT[:, b, h, jc * P : (jc + 1) * P],
                            qT[:, b, h, ib * IB : (ib + 1) * IB],
                            start=True,
                            stop=True,
                        )
                    nc.scalar.activation(
                        ptile[:, jg * 2 : (jg + 1) * 2, :],
                        ps_st,
                        mybir.ActivationFunctionType.Exp,
                        scale=SCALE,
                    )
                return ptile

            def attnv_block(h, b, ib, ptile):
                """attn@V with the denominator in column DH, then normalize
                and stage this block's A2A input."""
                po = h * DH
                ps_o4 = o_psum.tile(
                    [P, ISUB, DH + 1], FP32, tag="po", name="o_ps"
                )
                for isub in range(ISUB):
                    for jc in range(JC):
                        nc.tensor.matmul(
                            ps_o4[:, isub, :],
                            ptile[:, jc, isub * P : (isub + 1) * P],
                            v_aug[:, b, jc, h, :],
                            start=(jc == 0),
                            stop=(jc == JC - 1),
                        )
                for isub in range(ISUB):
                    ic = ib * ISUB + isub
                    recip = nrm.tile([P, 1], FP32, tag="recip", name="recip")
                    nc.vector.reciprocal(recip, ps_o4[:, isub, DH : DH + 1])
                    nc.vector.tensor_scalar_mul(
                        out_rows[:, b, ic, po : po + DH],
                        ps_o4[:, isub, 0:DH],
                        recip,
                    )
                s = b * NIB + ib
                nc.sync.dma_start(
                    a2a_ins[h][s],
                    out_rows[
                        :, b, ib * ISUB : (ib + 1) * ISUB, po : po + DH
                    ],
                )

            def a2a_exchange(h):
                nc.gpsimd.collective_compute(
                    "AllToAll",
                    mybir.AluOpType.bypass,
                    replica_groups=REPLICA_GROUPS,
                    ins=[a2a_ins[h].opt()],
                    outs=[a2a_outs[h].opt()],
                )

            def receive(hl, i_list):
                """Unpack a2a_outs[hl] blocks from source cores i_list into
                attnT chunk (hl*4 + i//2) at partition offset (i%2)*64."""
                rstages = {}
                for i in i_list:
                    rstage = rst_pool.tile(
                        [P, NIB, DH], BF16, tag="rstage", name="rstage"
                    )
                    nc.sync.dma_start(rstage, a2a_outs[hl][i])
                    rstages[i] = rstage
                for i in i_list:
                    rps = st_psum.tile([DH, NIB, P], BF16, tag="st", name="r_ps")
                    for q in range(NIB):
                        nc.tensor.transpose(
                            rps[:, q, :], rstages[i][:, q, :], ident_bf
                        )
                    pb = (i % 2) * DH
                    nc.vector.tensor_copy(
                        attnT[pb : pb + DH, hl * 4 + i // 2, :], rps
                    )

            def final_half_a(cc):
                """Output projection over attnT chunks 0-3 (head-local 0),
                bias folded in; parked in fstage."""
                psf2 = st_psum.tile([P, 2, IB], FP32, tag="st", name="fa_ps")
                ps_f = psf2[:, 0, :ROWS]
                for ko in range(4):
                    nc.tensor.matmul(
                        ps_f,
                        wo_sb[:, ko, cc * P : (cc + 1) * P],
                        attnT[:, ko, :],
                        start=(ko == 0),
                        stop=(ko == 3),
                    )
                nc.vector.tensor_scalar_add(
                    fstage[:, cc, :], ps_f, bias_sb[:, cc : cc + 1]
                )

            def final_half_b(cc):
                """Chunks 4-7 (head-local 1), summed with the parked half."""
                psf2 = st_psum.tile([P, 2, IB], FP32, tag="st", name="fb_ps")
                ps_f = psf2[:, 0, :ROWS]
                for ko in range(4, KO):
                    nc.tensor.matmul(
                        ps_f,
                        wo_sb[:, ko, cc * P : (cc + 1) * P],
                        attnT[:, ko, :],
                        start=(ko == 4),
                        stop=(ko == KO - 1),
                    )
                of = stage.tile([P, ROWS], FP32, tag="of", name="of")
                nc.vector.tensor_tensor(
                    of, ps_f, fstage[:, cc, :], mybir.AluOpType.add
                )
                nc.sync.dma_start(out_ext[cc * P : (cc + 1) * P, :], of)

            # ---- emission order: software-pipelined so the exp stream
            # starts early and never starves, with projection / receive /
            # half-projection work filling the PE's exp-lag bubbles ----
            # gpsimd queue order matters at startup: the qkv weight DMAs
            # must precede the big pad memsets (each ~6-10us on gpsimd) or
            # the first qk matmul waits ~20us for wk to land
            load_qkv_weights()
            nc.gpsimd.memset(qT[DH:P, :, :, :], 0.0)
            nc.gpsimd.memset(kT[DH:P, :, :, :], 0.0)
            xT0 = load_xT(0, engines=(nc.sync, nc.scalar))
            xT1 = load_xT(1)
            load_out_weights()
            make_identity(nc, ident_bf)

            for nb in range(NIB):
                qk_group(0, "k", nb, xT0)
            qk_group(0, "q", 0, xT0)

            pt = {}
            pt[(0, 0, 0)] = dots_block(0, 0, 0)
            qk_group(0, "q", 1, xT0)
            qk_group(0, "q", 2, xT0)
            pt[(0, 0, 1)] = dots_block(0, 0, 1)
            qk_group(0, "q", 3, xT0)
            for mt in range(8):
                v_group(0, mt, xT0)
            pt[(0, 0, 2)] = dots_block(0, 0, 2)
            for mt in range(8, JC):
                v_group(0, mt, xT0)
            pt[(0, 0, 3)] = dots_block(0, 0, 3)

            attnv_block(0, 0, 0, pt[(0, 0, 0)])
            qk_group(1, "k", 0, xT1)
            qk_group(1, "k", 1, xT1)
            attnv_block(0, 0, 1, pt[(0, 0, 1)])
            qk_group(1, "k", 2, xT1)
            qk_group(1, "k", 3, xT1)
            attnv_block(0, 0, 2, pt[(0, 0, 2)])
            qk_group(1, "q", 0, xT1)
            pt[(0, 1, 0)] = dots_block(0, 1, 0)
            attnv_block(0, 0, 3, pt[(0, 0, 3)])
            qk_group(1, "q", 1, xT1)
            pt[(0, 1, 1)] = dots_block(0, 1, 1)
            for mt in range(8):
                v_group(1, mt, xT1)
            attnv_block(0, 1, 0, pt[(0, 1, 0)])
            qk_group(1, "q", 2, xT1)
            pt[(0, 1, 2)] = dots_block(0, 1, 2)
            for mt in range(8, JC):
                v_group(1, mt, xT1)
            attnv_block(0, 1, 1, pt[(0, 1, 1)])
            qk_group(1, "q", 3, xT1)
            pt[(0, 1, 3)] = dots_block(0, 1, 3)
            attnv_block(0, 1, 2, pt[(0, 1, 2)])
            attnv_block(0, 1, 3, pt[(0, 1, 3)])
            a2a_exchange(0)

            pt[(1, 0, 0)] = dots_block(1, 0, 0)
            pt[(1, 0, 1)] = dots_block(1, 0, 1)
            attnv_block(1, 0, 0, pt[(1, 0, 0)])
            pt[(1, 0, 2)] = dots_block(1, 0, 2)
            attnv_block(1, 0, 1, pt[(1, 0, 1)])
            pt[(1, 0, 3)] = dots_block(1, 0, 3)
            attnv_block(1, 0, 2, pt[(1, 0, 2)])
            attnv_block(1, 0, 3, pt[(1, 0, 3)])

            pt[(1, 1, 0)] = dots_block(1, 1, 0)
            pt[(1, 1, 1)] = dots_block(1, 1, 1)
            attnv_block(1, 1, 0, pt[(1, 1, 0)])
            pt[(1, 1, 2)] = dots_block(1, 1, 2)
            attnv_block(1, 1, 1, pt[(1, 1, 1)])
            pt[(1, 1, 3)] = dots_block(1, 1, 3)
            attnv_block(1, 1, 2, pt[(1, 1, 2)])
            attnv_block(1, 1, 3, pt[(1, 1, 3)])
            a2a_exchange(1)
            # head-0 receive + first half of the output projection fill
            # the PE while the head-1 A2A is in flight
            receive(0, [0, 1, 2, 3])
            receive(0, [4, 5, 6, 7])
            for cc in range(KO):
                final_half_a(cc)
            receive(1, list(range(NCORES)))
            for cc in range(KO):
                final_half_b(cc)

    nc.finalize()
    return nc


def _get_nc():
    if "nc" not in _NC_CACHE:
        _NC_CACHE["nc"] = _build()
    return _NC_CACHE["nc"]


def _wo_row_perm():
    """attnT contraction chunk ko holds (local head ko//4, source-core
    pair ko%4) with partition p = (i%2)*64 + d; W_out rows are permuted
    to match so wo_sb[(ko, p)] multiplies the right inner dim."""
    perm = np.empty(DIM, dtype=np.int64)
    for ko in range(KO):
        hl, ipair = ko // 4, ko % 4
        for p_ in range(P):
            i = ipair * 2 + p_ // DH
            d = p_ % DH
            perm[ko * P + p_] = (2 * i + hl) * DH + d
    return perm


def kernel(**inputs) -> np.ndarray:
    import os

    import ml_dtypes

    global LAST_RESULTS

    bf16 = ml_dtypes.bfloat16
    x = np.asarray(inputs["x"], dtype=np.float32)
    W_qkv = np.asarray(inputs["W_qkv"], dtype=np.float32)
    W_out = np.asarray(inputs["W_out"], dtype=np.float32)
    b_out = np.ascontiguousarray(np.asarray(inputs["b_out"], dtype=np.float32))

    def _kpko(w):
        """[DIM, C] -> [P, KO*C] so on-chip wq/wk/wv/wo DMAs are fully
        contiguous: out[kp, ko*C + c] = w[ko*P + kp, c]."""
        cdim = w.shape[1]
        return np.ascontiguousarray(
            w.reshape(KO, P, cdim).transpose(1, 0, 2).reshape(P, KO * cdim)
        )

    x_bf = np.ascontiguousarray(
        x.transpose(0, 2, 1).reshape(B * DIM, N).astype(bf16)
    )
    wo_bf = _kpko(W_out.astype(bf16)[_wo_row_perm(), :])
    bo_r = np.ascontiguousarray(b_out.reshape(KO, P).T)
    wqkv_bf = W_qkv.astype(bf16)

    nc = _get_nc()

    in_maps = []
    for c in range(NCORES):
        in_maps.append(
            {
                "x": x_bf,
                "wq": _kpko(
                    wqkv_bf[:, 0 * INNER + c * SH : 0 * INNER + (c + 1) * SH]
                ),
                "wk": _kpko(
                    wqkv_bf[:, 1 * INNER + c * SH : 1 * INNER + (c + 1) * SH]
                ),
                "wv": _kpko(
                    wqkv_bf[:, 2 * INNER + c * SH : 2 * INNER + (c + 1) * SH]
                ),
                "wo": wo_bf,
                "bo": bo_r,
            }
        )

    trace = os.environ.get("BASS_KERNEL_TRACE", "0") == "1"
    res = run_bass_kernel_spmd(
        nc, in_maps, core_ids=list(range(NCORES)), trace=trace
    )
    LAST_RESULTS = res

    y = np.empty((B, N, DIM), dtype=np.float32)
    for c in range(NCORES):
        b, r = c // 4, c % 4
        y[b, r * ROWS : (r + 1) * ROWS, :] = res.results[c]["out"].T
    return y
